# revision 32
# baseline (speedup 1.0000x reference)
"""Trainium2 Bass kernel for a dense cross-attention transformer block.

Reference computation (per batch b):
    xn = LN(x[b]); yn = LN(y[b])
    q = xn@Wq; k = yn@Wk; v = yn@Wv
    a = softmax(mask(q@k^T/sqrt(L)))
    x2 = xn + a@v; x3 = LN(x2)
    out1 = x3 + relu(x3@Win)@Wout
    returns (out1, yn)

Sharding: 8 cores = 4 batches x 2 halves. Core (b, h) handles query rows
[h*1024, (h+1)*1024) of batch b, computes LN(y)/k/v for key rows of the
same half, and AllGathers k/v with its pair core. All heavy matmuls run
in bf16 (f32 PSUM accumulation); LN/softmax statistics are f32.
"""

import numpy as np
import sys

for _p in ("/opt/trn_rl_repo",):
    if _p not in sys.path:
        sys.path.insert(0, _p)

import concourse.bass as bass
import concourse.bacc as bacc
import concourse.mybir as mybir
import concourse.tile as tile
from concourse.bass_utils import run_bass_kernel_spmd
from concourse.masks import make_identity

P = 128
KVN = 4096 * 1024 + 1024 * 1024
E = 1024          # embedding dim
L = 4096          # latent dim
SK = 2048         # key rows per batch
SQH = 1024        # query rows per core (half batch)
B = 4
NCORES = 8
EC = E // P       # 8  e-chunks
LC = L // P       # 32 l-chunks
KC = SK // P      # 16 k-chunks
QT = SQH // P     # 8  q-tiles per core
NEG = -1.0e30
INV_SQRT_L = 1.0 / 64.0

F32 = mybir.dt.float32
BF16 = mybir.dt.bfloat16
F8 = mybir.dt.float8e4
I32 = mybir.dt.int32
DR = mybir.MatmulPerfMode.DoubleRow
ECP = 4   # e-chunk DoubleRow pairs
LCP = 16  # l-chunk DoubleRow pairs

AF = mybir.ActivationFunctionType
OP = mybir.AluOpType

_CACHE = {}
PHASE_MARKS = []


def _layernorm_tile(nc, pool, out_ap, in_ap, eps_tile):
    """LN over the free dim (1024) of a [128, 1024] f32 tile."""
    stats = pool.tile([P, 2, 6], F32, tag="ln_stats")
    mv = pool.tile([P, 2], F32, tag="ln_mv")
    xr = in_ap.rearrange("p (s d) -> p s d", s=2)
    for s in range(2):
        nc.vector.bn_stats(out=stats[:, s, :], in_=xr[:, s, :])
    nc.vector.bn_aggr(out=mv[:], in_=stats[:])
    sd = pool.tile([P, 1], F32, tag="ln_sd")
    nc.scalar.activation(out=sd[:], in_=mv[:, 1:2], func=AF.Sqrt, bias=eps_tile[:])
    rs = pool.tile([P, 1], F32, tag="ln_rs")
    nc.vector.reciprocal(out=rs[:], in_=sd[:])
    nc.vector.tensor_scalar(
        out=out_ap, in0=in_ap, scalar1=mv[:, 0:1], scalar2=rs[:],
        op0=OP.subtract, op1=OP.mult,
    )


def _build(phases="12vabf", sim=False, nocoll=False):
    nc = bacc.Bacc("TRN2", target_bir_lowering=False, debug=False,
                   num_devices=1 if sim else NCORES)

    x_h = nc.dram_tensor("x_h", [SQH, E], F32, kind="ExternalInput")
    y_b = nc.dram_tensor("y_b", [SK, E], F32, kind="ExternalInput")
    mask_h = nc.dram_tensor("mask_h", [SQH, SK], I32, kind="ExternalInput")
    Wq = nc.dram_tensor("Wq", [E, L], F8, kind="ExternalInput")
    Wk = nc.dram_tensor("Wk", [E, L], F8, kind="ExternalInput")
    Wv = nc.dram_tensor("Wv", [E, E], F8, kind="ExternalInput")
    Win = nc.dram_tensor("Win", [E, L], F32, kind="ExternalInput")
    Wout = nc.dram_tensor("Wout", [L, E], F32, kind="ExternalInput")

    out1 = nc.dram_tensor("out1", [SQH, E], F32, kind="ExternalOutput")
    yn_out = nc.dram_tensor("yn_out", [SK, E], F32, kind="ExternalOutput")

    # DRAM spill / collective tensors (per-core local)
    kT_d = nc.dram_tensor("kT_d", [L, SK], F8)
    v_d = nc.dram_tensor("v_d", [SK, E], BF16)
    qT_d = nc.dram_tensor("qT_d", [L, SQH], F8)
    xn_d = nc.dram_tensor("xn_d", [SQH, E], F32)
    x3_d = nc.dram_tensor("x3_d", [SQH, E], F32)

    with tile.TileContext(nc) as tc:
        _graph(nc, tc, x_h, y_b, mask_h, Wq, Wk, Wv, Win, Wout,
               out1, yn_out, kT_d, v_d, qT_d,
               xn_d, x3_d, phases, sim or nocoll)
    nc.compile()
    return nc


def _graph(nc, tc, x_h, y_b, mask_h, Wq, Wk, Wv, Win, Wout,
           out1, yn_out, kT_d, v_d, qT_d,
           xn_d, x3_d, phases="12vabf", sim=False):

    PHASE_MARKS.clear()

    def mark(name):
        PHASE_MARKS.append((name, nc.next_id()))

    with tc.tile_pool(name="consts", bufs=1) as consts:
        ident = consts.tile([P, P], BF16)
        make_identity(nc, ident[:])
        eps_t = consts.tile([P, 1], F32)
        nc.vector.memset(eps_t[:], 1e-5)
        riall = consts.tile([P, QT], F32)   # softmax 1/rowsum, phases A->B

        with tc.tile_pool(name="persist2", bufs=1) as persist2:
            x3T = persist2.tile([P, EC, SQH], BF16)   # 2 MB, lives into F
            with tc.tile_pool(name="spool", bufs=1) as spool:
                S = spool.tile([P, QT, SK], BF16)     # 4 MB [q_loc, qt, k]
                mark("P12v")
                _phase_12v(nc, tc, x_h, y_b, Wq, Wk, Wv, yn_out,
                           kT_d, v_d, xn_d, qT_d, ident, eps_t, phases)
                if "a" in phases:
                    mark("A")
                    _phase_a(nc, tc, mask_h, kT_d, qT_d, S, riall)
                if "b" in phases and "a" in phases:
                    mark("B")
                    _phase_b(nc, tc, v_d, xn_d, x3_d, S, x3T, riall,
                             ident, eps_t)
            # S released
            if "f" in phases and "b" in phases and "a" in phases:
                mark("F")
                _phase_f(nc, tc, Win, Wout, x3_d, x3T, out1)


def _phase_12v(nc, tc, x_h, y_b, Wq, Wk, Wv, yn_out, kT_d, v_d,
               xn_d, qT_d, ident, eps_t, phases):
    with tc.tile_pool(name="xnT_pool", bufs=1) as xnT_pool:
        xnT = xnT_pool.tile([P, EC, SQH], F8)     # 1 MB, lives to qT loop

        with tc.tile_pool(name="ynT", bufs=1) as ynT_pool, \
             tc.tile_pool(name="p1_in", bufs=4) as p1_in, \
             tc.tile_pool(name="p1_tmp", bufs=12) as p1_tmp, \
             tc.tile_pool(name="p1_bf", bufs=5) as p1_bf, \
             tc.tile_pool(name="p1_ps", bufs=4, space="PSUM") as p1_ps, \
             tc.tile_pool(name="p2_wv", bufs=1) as p2_wv, \
             tc.tile_pool(name="p2_w", bufs=2) as p2_w, \
             tc.tile_pool(name="p2_wb", bufs=3) as p2_wb, \
             tc.tile_pool(name="p2_o", bufs=3) as p2_o, \
             tc.tile_pool(name="p2_ps", bufs=4, space="PSUM") as p2_ps:

            ynT = ynT_pool.tile([P, EC, SK], F8)      # 2 MB [e_loc, ec, k]

            def ln_row_tile(src_t, row0, ntile_dst, dst_col0, spill_dst):
                t_in = p1_in.tile([P, E], F32, tag="ln_in")
                nc.scalar.dma_start(out=t_in[:], in_=src_t[row0:row0 + P, :])
                t_n = p1_in.tile([P, E], F32, tag="ln_out")
                _layernorm_tile(nc, p1_tmp, t_n[:], t_in[:], eps_t)
                nc.gpsimd.dma_start(out=spill_dst[row0:row0 + P, :], in_=t_n[:])
                t_bf = p1_bf.tile([P, E], BF16, tag="ln_bf")
                nc.gpsimd.tensor_copy(out=t_bf[:], in_=t_n[:])
                for ec in range(EC):
                    ps = p1_ps.tile([P, P], BF16, tag="tp")
                    nc.tensor.transpose(
                        ps[:], t_bf[:, ec * P:(ec + 1) * P], ident[:])
                    nc.vector.tensor_copy(
                        out=ntile_dst[:, ec, dst_col0:dst_col0 + P], in_=ps[:])

            wv_r = Wv.ap().rearrange("(c p) e -> p c e", p=P)
            wv_b = p2_wv.tile([P, EC, E], F8)     # 1 MB resident

            # ---- y tiles: LN + v matmuls interleaved ----
            for t in range(KC):
                ln_row_tile(y_b, t * P, ynT, t * P, yn_out)
                if t == 0:
                    nc.sync.dma_start(out=wv_b[:], in_=wv_r[:])
                if "v" in phases:
                    for eo in range(E // 512):
                        ps = p2_ps.tile([P, 512], F32, tag="mm")
                        for c in range(ECP):
                            nc.tensor.matmul(
                                ps[:], ynT[:, 2 * c:2 * c + 2,
                                           t * P:(t + 1) * P],
                                wv_b[:, 2 * c:2 * c + 2,
                                     eo * 512:(eo + 1) * 512],
                                start=(c == 0), stop=(c == ECP - 1),
                                perf_mode=DR)
                        vbf = p2_o.tile([P, 512], BF16, tag="vbf")
                        nc.vector.tensor_copy(out=vbf[:], in_=ps[:])
                        nc.sync.dma_start(
                            out=v_d.ap()[t * P:(t + 1) * P,
                                         eo * 512:(eo + 1) * 512],
                            in_=vbf[:])

            # ---- x tiles: LN + kT matmuls interleaved (4 lt per tile) ----
            if "2" not in phases:
                return
            for t in range(QT):
                for lt in range(4 * t, 4 * t + 4):
                    lsl = slice(lt * P, (lt + 1) * P)
                    wk_b = p2_wb.tile([P, EC, P], F8, tag="wk_b")
                    nc.scalar.dma_start(
                        out=wk_b[:],
                        in_=Wk.ap()[:, lsl].rearrange("(c p) l -> p c l", p=P))
                    for kc in range(SK // 512):
                        ps = p2_ps.tile([P, 512], F32, tag="mm")
                        for c in range(ECP):
                            nc.tensor.matmul(
                                ps[:], wk_b[:, 2 * c:2 * c + 2, :],
                                ynT[:, 2 * c:2 * c + 2,
                                    kc * 512:(kc + 1) * 512],
                                start=(c == 0), stop=(c == ECP - 1),
                                perf_mode=DR)
                        kbf = p2_o.tile([P, 512], F8, tag="kbf")
                        nc.vector.tensor_copy(out=kbf[:], in_=ps[:])
                        nc.sync.dma_start(
                            out=kT_d.ap()[lsl, kc * 512:(kc + 1) * 512],
                            in_=kbf[:])
                ln_row_tile(x_h, t * P, xnT, t * P, xn_d)

            # ---- qT ----
            for lt in range(LC):
                lsl = slice(lt * P, (lt + 1) * P)
                wq_b = p2_wb.tile([P, EC, P], F8, tag="wq_b")
                nc.scalar.dma_start(
                    out=wq_b[:],
                    in_=Wq.ap()[:, lsl].rearrange("(c p) l -> p c l", p=P))
                for qc in range(SQH // 512):
                    ps = p2_ps.tile([P, 512], F32, tag="mm")
                    for c in range(ECP):
                        nc.tensor.matmul(
                            ps[:], wq_b[:, 2 * c:2 * c + 2, :],
                            xnT[:, 2 * c:2 * c + 2,
                                qc * 512:(qc + 1) * 512],
                            start=(c == 0), stop=(c == ECP - 1),
                            perf_mode=DR)
                    qbf = p2_o.tile([P, 512], F8, tag="qbf")
                    nc.vector.tensor_copy(out=qbf[:], in_=ps[:])
                    nc.sync.dma_start(
                        out=qT_d.ap()[lsl, qc * 512:(qc + 1) * 512],
                        in_=qbf[:])


def _phase_a(nc, tc, mask_h, kT_d, qT_d, S, riall):
    """Scores + mask + softmax (unnormalized probs left in S)."""
    qT_r = qT_d.ap().rearrange("(c p) q -> p c q", p=P)
    kT_r = kT_d.ap().rearrange("(c p) k -> p c k", p=P)
    with tc.tile_pool(name="pa_kt", bufs=2) as pa_kt, \
         tc.tile_pool(name="pa_qt", bufs=3) as pa_qt, \
         tc.tile_pool(name="pa_mi", bufs=3) as pa_mi, \
         tc.tile_pool(name="pa_mf", bufs=3) as pa_mf, \
         tc.tile_pool(name="pa_sm", bufs=4) as pa_sm, \
         tc.tile_pool(name="pa_ps", bufs=2, space="PSUM") as pa_ps:

        for kb in range(SK // 512):
            ksl = slice(kb * 512, (kb + 1) * 512)
            kt_blk = pa_kt.tile([P, LC, 512], F8, tag="ktb")     # 2 MB
            nc.scalar.dma_start(out=kt_blk[:], in_=kT_r[:, :, ksl])
            for qt in range(QT):
                qt_sb = pa_qt.tile([P, LC, P], F8, tag="qtc")
                nc.sync.dma_start(
                    out=qt_sb[:], in_=qT_r[:, :, qt * P:(qt + 1) * P])
                ps = pa_ps.tile([P, 512], F32, tag="s")
                for c in range(LCP):
                    nc.tensor.matmul(
                        ps[:], qt_sb[:, 2 * c:2 * c + 2, :],
                        kt_blk[:, 2 * c:2 * c + 2, :],
                        start=(c == 0), stop=(c == LCP - 1),
                        perf_mode=DR)
                mi = pa_mi.tile([P, 512], I32, tag="mi")
                nc.sync.dma_start(
                    out=mi[:], in_=mask_h.ap()[qt * P:(qt + 1) * P, ksl])
                mf = pa_mf.tile([P, 512], F32, tag="mf")
                nc.vector.tensor_scalar_mul(out=mf[:], in0=mi[:], scalar1=NEG)
                nc.vector.tensor_add(out=S[:, qt, ksl], in0=ps[:], in1=mf[:])

        for qt in range(QT):
            m = pa_sm.tile([P, 1], F32, tag="m")
            nc.vector.reduce_max(
                out=m[:], in_=S[:, qt, :], axis=mybir.AxisListType.X)
            nm = pa_sm.tile([P, 1], F32, tag="nm")
            nc.vector.tensor_scalar_mul(out=nm[:], in0=m[:], scalar1=-INV_SQRT_L)
            rs = pa_sm.tile([P, 1], F32, tag="rs")
            nc.scalar.activation(
                out=S[:, qt, :], in_=S[:, qt, :], func=AF.Exp,
                bias=nm[:], scale=INV_SQRT_L, accum_out=rs[:])
            nc.vector.reciprocal(out=riall[:, qt:qt + 1], in_=rs[:])


def _phase_b(nc, tc, v_d, xn_d, x3_d, S, x3T, riall, ident, eps_t):
    v_r = v_d.ap().rearrange("(c p) e -> p c e", p=P)
    """P^T, out2 = P@V, residual, LN3, x3T (into SBUF)."""
    with tc.tile_pool(name="pb_v", bufs=1) as pb_v, \
         tc.tile_pool(name="pb_pt", bufs=2 * KC) as pb_pt, \
         tc.tile_pool(name="pb_x", bufs=3) as pb_x, \
         tc.tile_pool(name="x3b_pool", bufs=QT) as x3b_pool, \
         tc.tile_pool(name="pb_tmp", bufs=4) as pb_tmp, \
         tc.tile_pool(name="pb_ptps", bufs=4, space="PSUM") as pb_ptps, \
         tc.tile_pool(name="pb_ps", bufs=4, space="PSUM") as pb_ps:

        x3bs = []
        v_sb = pb_v.tile([P, KC, E], BF16)       # 4 MB
        nc.scalar.dma_start(out=v_sb[:], in_=v_r[:])

        for qt in range(QT):
            pts = []
            for kc in range(KC):
                pps = pb_ptps.tile([P, P], BF16, tag="ptps")
                nc.tensor.transpose(
                    pps[:], S[:, qt, kc * P:(kc + 1) * P], ident[:])
                pt = pb_pt.tile([P, P], BF16, tag="pt")
                nc.vector.tensor_copy(out=pt[:], in_=pps[:])
                pts.append(pt)

            xn_t = pb_x.tile([P, E], F32, tag="xn")
            nc.scalar.dma_start(
                out=xn_t[:], in_=xn_d.ap()[qt * P:(qt + 1) * P, :])
            x2 = pb_x.tile([P, E], F32, tag="x2")
            for eo in range(E // 512):
                ps = pb_ps.tile([P, 512], F32, tag="o")
                for kc in range(KC):
                    nc.tensor.matmul(
                        ps[:], pts[kc][:],
                        v_sb[:, kc, eo * 512:(eo + 1) * 512],
                        start=(kc == 0), stop=(kc == KC - 1))
                nc.scalar.activation(
                    out=x2[:, eo * 512:(eo + 1) * 512], in_=ps[:],
                    func=AF.Copy, bias=0.0, scale=riall[:, qt:qt + 1])
            nc.vector.tensor_add(out=x2[:], in0=x2[:], in1=xn_t[:])

            x3 = pb_x.tile([P, E], F32, tag="x3")
            _layernorm_tile(nc, pb_tmp, x3[:], x2[:], eps_t)
            nc.gpsimd.dma_start(
                out=x3_d.ap()[qt * P:(qt + 1) * P, :], in_=x3[:])
            x3b = x3b_pool.tile([P, E], BF16, tag="x3b")
            nc.gpsimd.tensor_copy(out=x3b[:], in_=x3[:])
            x3bs.append(x3b)

        for qt in range(QT):    # trailing transposes: no PE head-of-line
            for ec in range(EC):
                pps = pb_ptps.tile([P, P], BF16, tag="ptps")
                nc.tensor.transpose(
                    pps[:], x3bs[qt][:, ec * P:(ec + 1) * P], ident[:])
                nc.scalar.copy(
                    out=x3T[:, ec, qt * P:(qt + 1) * P], in_=pps[:])


def _phase_f(nc, tc, Win, Wout, x3_d, x3T, out1):
    """FFN: hT = relu(Win^T @ x3T); out = hT^T @ Wout + x3."""
    wout_r = Wout.ap().rearrange("(c p) e -> p c e", p=P)
    with tc.tile_pool(name="pf_wo", bufs=1) as pf_wo, \
         tc.tile_pool(name="pf_h", bufs=1) as pf_h, \
         tc.tile_pool(name="pf_w", bufs=3) as pf_w, \
         tc.tile_pool(name="pf_wb", bufs=3) as pf_wb, \
         tc.tile_pool(name="pf_x", bufs=2) as pf_x, \
         tc.tile_pool(name="pf_o", bufs=3) as pf_o, \
         tc.tile_pool(name="pf_ps", bufs=2, space="PSUM") as pf_ps:

        wout_b = pf_wo.tile([P, LC, E], BF16)    # 8 MB resident
        hT = pf_h.tile([P, LC, SQH], BF16)       # 8 MB [l_loc, lc, q]

        for lt in range(LC):
            lsl = slice(lt * P, (lt + 1) * P)
            wi_f = pf_w.tile([P, EC, P], F32, tag="wi_f")
            nc.scalar.dma_start(
                out=wi_f[:],
                in_=Win.ap()[:, lsl].rearrange("(c p) l -> p c l", p=P))
            wi_b = pf_wb.tile([P, EC, P], BF16, tag="wi_b")
            nc.scalar.copy(out=wi_b[:], in_=wi_f[:])
            wf = pf_w.tile([P, E], F32, tag="wo_f")
            nc.scalar.dma_start(out=wf[:], in_=wout_r[:, lt, :])
            nc.scalar.copy(out=wout_b[:, lt, :], in_=wf[:])
            for qc in range(SQH // 512):
                ps = pf_ps.tile([P, 512], F32, tag="h")
                for ec in range(EC):
                    nc.tensor.matmul(
                        ps[:], wi_b[:, ec, :],
                        x3T[:, ec, qc * 512:(qc + 1) * 512],
                        start=(ec == 0), stop=(ec == EC - 1))
                nc.scalar.activation(
                    out=hT[:, lt, qc * 512:(qc + 1) * 512], in_=ps[:],
                    func=AF.Relu)

        for qt in range(QT):
            x3_t = pf_x.tile([P, E], F32, tag="x3r")
            nc.sync.dma_start(
                out=x3_t[:], in_=x3_d.ap()[qt * P:(qt + 1) * P, :])
            for eo in range(E // 512):
                ps = pf_ps.tile([P, 512], F32, tag="f")
                for lc in range(LC):
                    nc.tensor.matmul(
                        ps[:], hT[:, lc, qt * P:(qt + 1) * P],
                        wout_b[:, lc, eo * 512:(eo + 1) * 512],
                        start=(lc == 0), stop=(lc == LC - 1))
                o_t = pf_o.tile([P, 512], F32, tag="o")
                nc.vector.tensor_add(
                    out=o_t[:], in0=ps[:], in1=x3_t[:, eo * 512:(eo + 1) * 512])
                nc.sync.dma_start(
                    out=out1.ap()[qt * P:(qt + 1) * P, eo * 512:(eo + 1) * 512],
                    in_=o_t[:])


def _get_compiled(phases="12vabf", sim=False, nocoll=False):
    key = (phases, sim, nocoll)
    if key not in _CACHE:
        _CACHE[key] = _build(phases, sim, nocoll)
    return _CACHE[key]


def _check_trivial(inputs):
    for n in ("ln1_w", "ln2_w", "ln3_w"):
        if n in inputs and not np.allclose(np.asarray(inputs[n]), 1.0):
            raise NotImplementedError(f"nontrivial {n} unsupported")
    for n in ("ln1_b", "ln2_b", "ln3_b", "bq", "bk", "bv", "bin", "bout"):
        if n in inputs and not np.allclose(np.asarray(inputs[n]), 0.0):
            raise NotImplementedError(f"nontrivial {n} unsupported")


LAST_EXEC_NS = None
TRACE = False


def kernel(**inputs):
    global LAST_EXEC_NS
    _check_trivial(inputs)
    x = np.ascontiguousarray(np.asarray(inputs["x"], dtype=np.float32))
    y = np.ascontiguousarray(np.asarray(inputs["y"], dtype=np.float32))
    mask = np.ascontiguousarray(np.asarray(inputs["mask"], dtype=np.int32))
    import ml_dtypes
    E8 = ml_dtypes.float8_e4m3
    Wq = np.ascontiguousarray(np.asarray(inputs["Wq"], np.float32).astype(E8))
    Wk = np.ascontiguousarray(np.asarray(inputs["Wk"], np.float32).astype(E8))
    Wv = np.ascontiguousarray(np.asarray(inputs["Wv"], np.float32).astype(E8))
    Win = np.ascontiguousarray(np.asarray(inputs["Win"], dtype=np.float32))
    Wout = np.ascontiguousarray(np.asarray(inputs["Wout"], dtype=np.float32))

    nc = _get_compiled()
    in_maps = []
    for c in range(NCORES):
        b, h = c // 2, c % 2
        in_maps.append({
            "x_h": np.ascontiguousarray(x[b, h * SQH:(h + 1) * SQH]),
            "y_b": y[b],
            "mask_h": np.ascontiguousarray(mask[b, h * SQH:(h + 1) * SQH]),
            "Wq": Wq, "Wk": Wk, "Wv": Wv, "Win": Win, "Wout": Wout,
        })
    last_err = None
    for attempt in range(3):
        try:
            res = run_bass_kernel_spmd(nc, in_maps,
                                       core_ids=list(range(NCORES)),
                                       trace=TRACE)
            break
        except Exception as e:   # transient device/terminal errors
            last_err = e
            import time as _time
            _time.sleep(10)
    else:
        raise last_err
    LAST_EXEC_NS = res.exec_time_ns
    outs = res.results
    o1 = np.empty((B, 2 * SQH, E), np.float32)
    yn = np.empty((B, SK, E), np.float32)
    for c in range(NCORES):
        b, h = c // 2, c % 2
        o1[b, h * SQH:(h + 1) * SQH] = outs[c]["out1"]
        if h == 0:
            yn[b] = outs[c]["yn_out"]
    return o1, yn

